# revision 1
# baseline (speedup 1.0000x reference)
"""Trainium2 Bass kernel for nn_GCN2 (Chebyshev feature GCN, 8 NeuronCores).

Math (matching the reference):
    T0 = X;  T1 = L X;  T2 = 2 L T1 - T0
    h1 = T0 W1a + T1 W1b + T2 W1c    (feats @ W1 without materializing concat)
       = X (W1a - W1c) + L (X W1b) + 2 L (L (X W1c))
    x_l = relu(S h_l + b_l),  S = D^-1/2 (A + I) D^-1/2,  h_l = x_{l-1} W_l
    out = softmax(((x1+x2+x3)/3 pooled-by-graph-mean) Wout + bout)

Because row scaling commutes with right multiplication, the gathered tables
are pre-scaled:  S h = dis * (A+I)-segment-sum( dis * h ), so each conv
gathers rows of (dis * x W) and post-scales the segment sum by dis.

Sharding: nodes are permuted (degree-sorted, dealt round-robin to 8 cores so
every core owns 49 destination tiles of 128 nodes with nearly equal work).
Every SpMM is a padded dest-major gather: for each tile of 128 destination
nodes the incident edges' source rows are gathered with dma_gather (dest d's
j-th edge lands at partition d, free slot j), then a strided tensor_reduce
sums the per-destination slots.  dma_gather indices are int16, so gathers
from the 50176-row tables are split into lo/hi half-table rectangles.
Feature tables are AllGather'ed between hops.
"""

import os
import numpy as np

import concourse.bass as bass
import concourse.bacc as bacc
import concourse.mybir as mybir
import concourse.tile as tile
from concourse.bass_utils import run_bass_kernel_spmd
from concourse.masks import make_identity

# ---- problem geometry (fixed for this problem) ----
N = 50000
DIN = 128
F = 64
NG = 512
DOUT = 10

NCORES = 8
P = 128
TPC = 49                  # dest tiles per core
NPC = TPC * P             # node rows per core (6272)
NTOT = NCORES * NPC       # padded rows (50176)
VIEW = 32768              # int16-addressable rows per table view
HI_BASE = NTOT - VIEW     # hi view = rows [17408, 50176)
# edges with src in the overlap [HI_BASE, VIEW) can use either view
PADROW_LO = 6250          # core 0's first dummy row (zero in every table)
PADROW_HI = 3 * NPC + 6250 - HI_BASE   # core 3's dummy, rebased into hi view

F32 = mybir.dt.float32
I16 = mybir.dt.int16

# gather chunk caps (slots per partition in one dma_gather call)
CAP_LO_W128 = 48
CAP_HI_W128 = 24
CAP_LO_W64 = 56
CAP_HI_W64 = 32

LAST_EXEC_TIME_NS = None


# ---------------- host-side graph preprocessing ----------------

def _perm_from_indeg(indeg):
    order = np.argsort(-indeg, kind="stable").astype(np.int64)
    perm = np.full(NTOT, -1, np.int64)
    for c in range(NCORES):
        own = order[c::NCORES]
        perm[c * NPC: c * NPC + own.size] = own
    inv = np.full(N, -1, np.int64)
    mask = perm >= 0
    inv[perm[mask]] = np.nonzero(mask)[0]
    return perm, inv


def _wrap16(flat_i32):
    """Pack a flat gather stream for dma_gather: idx i -> [i%16, i//16],
    replicated to the 8 groups of 16 partitions."""
    n = flat_i32.size
    assert n % 16 == 0
    a = flat_i32.reshape(n // 16, 16).T.astype(np.int16)
    return np.ascontiguousarray(np.tile(a, (8, 1)))


def _build_stage(drow, srow, vals):
    """Build per-core padded gather streams for one SpMM stage.

    drow: per-edge destination row (perm space of the *output*).
    srow: per-edge source row (perm space of the gathered table).
    vals: per-edge weights or None.

    Returns dict with compile-time DEG_LO/DEG_HI [TPC] (shared across cores)
    and per-core packed idx (int16 wrapped) / value arrays.
    """
    o = np.lexsort((srow, drow))
    d = drow[o]
    s = srow[o]
    v = vals[o] if vals is not None else None

    counts = np.bincount(d, minlength=NTOT)
    ptr = np.zeros(NTOT + 1, np.int64)
    np.cumsum(counts, out=ptr[1:])

    # categories: 0 = must-lo (< HI_BASE), 1 = flex (overlap), 2 = must-hi
    cat = np.where(s < HI_BASE, 0, np.where(s < VIEW, 1, 2)).astype(np.int64)
    n0 = np.bincount(d[cat == 0], minlength=NTOT)
    n1 = np.bincount(d[cat == 1], minlength=NTOT)
    n2 = np.bincount(d[cat == 2], minlength=NTOT)
    # assign t flex edges to lo to balance per-dest lo/hi loads
    t = np.clip((n1 + n2 - n0 + 1) // 2, 0, n1)

    # per-edge rank within (dest, category)
    ends = {}
    for cc in (0, 1, 2):
        m = cat == cc
        cptr = np.zeros(NTOT + 1, np.int64)
        np.cumsum(np.bincount(d[m], minlength=NTOT), out=cptr[1:])
        r = np.zeros(d.size, np.int64)
        r[m] = np.arange(m.sum(), dtype=np.int64) - cptr[d[m]]
        ends[cc] = r
    crank = ends[0] + ends[1] + ends[2]  # rank within own category

    flex_lo = (cat == 1) & (crank < t[d])
    is_lo = (cat == 0) | flex_lo
    jslot = np.where(cat == 0, crank,
                     np.where(flex_lo, n0[d] + crank,
                              np.where(cat == 1, n2[d] + (crank - t[d]),
                                       crank)))
    locnt = n0 + t
    hicnt = counts - locnt

    DEG_LO = locnt.reshape(NCORES, TPC, P).max(axis=(0, 2)).astype(np.int64)
    DEG_HI = hicnt.reshape(NCORES, TPC, P).max(axis=(0, 2)).astype(np.int64)
    CUM_LO = np.zeros(TPC + 1, np.int64)
    np.cumsum(DEG_LO, out=CUM_LO[1:])
    CUM_HI = np.zeros(TPC + 1, np.int64)
    np.cumsum(DEG_HI, out=CUM_HI[1:])
    SL, SH = int(CUM_LO[-1]), int(CUM_HI[-1])

    core = d // NPC
    b = (d % NPC) // P
    dslot = d % P

    idx_lo = np.full((NCORES, SL * P), PADROW_LO, np.int32)
    idx_hi = np.full((NCORES, SH * P), PADROW_HI, np.int32)
    pos_lo = (CUM_LO[b] + jslot) * P + dslot
    pos_hi = (CUM_HI[b] + jslot) * P + dslot
    idx_lo[core[is_lo], pos_lo[is_lo]] = s[is_lo].astype(np.int32)
    hi = ~is_lo
    idx_hi[core[hi], pos_hi[hi]] = (s[hi] - HI_BASE).astype(np.int32)

    out = {
        "DEG_LO": DEG_LO, "DEG_HI": DEG_HI,
        "CUM_LO": CUM_LO, "CUM_HI": CUM_HI, "SL": SL, "SH": SH,
        "idx_lo": [_wrap16(idx_lo[c]) for c in range(NCORES)],
        "idx_hi": [_wrap16(idx_hi[c]) for c in range(NCORES)],
    }
    if v is not None:
        v_lo = np.zeros((NCORES, P, SL), np.float32)
        v_hi = np.zeros((NCORES, P, SH), np.float32)
        v_lo[core[is_lo], dslot[is_lo], CUM_LO[b[is_lo]] + jslot[is_lo]] = v[is_lo]
        v_hi[core[hi], dslot[hi], CUM_HI[b[hi]] + jslot[hi]] = v[hi]
        out["v_lo"] = [np.ascontiguousarray(v_lo[c]) for c in range(NCORES)]
        out["v_hi"] = [np.ascontiguousarray(v_hi[c]) for c in range(NCORES)]
    return out


def _chunks(DEG, cap):
    """Group tile indices into chunks with sum(DEG) <= cap (compile-time)."""
    res = []
    start, tot = 0, 0
    for i in range(TPC):
        dd = int(DEG[i])
        if tot + dd > cap and i > start:
            res.append((start, i))
            start, tot = i, 0
        tot += dd
    res.append((start, TPC))
    return res


def _prep(X, L_indices, L_values, batch, W1, W2, W3, Wout, b1, b2, b3, bout):
    """All host preprocessing; returns (in_maps, meta)."""
    Ls, Ld = L_indices[1].astype(np.int64), L_indices[0].astype(np.int64)
    Arow, Acol = L_indices[0].astype(np.int64), L_indices[1].astype(np.int64)

    # GCN degrees (with self loop), dis = rsqrt(deg)
    deg = np.bincount(Acol, minlength=N).astype(np.float64) + 1.0
    dis = (1.0 / np.sqrt(deg)).astype(np.float32)

    indeg_L = np.bincount(Ld, minlength=N)
    indeg_A = np.bincount(Acol, minlength=N) + 1
    permL, invL = _perm_from_indeg(indeg_L)
    permA, invA = _perm_from_indeg(indeg_A)

    # L stage: dest=L_indices[0], src=L_indices[1]; both in permL space.
    stL = _build_stage(invL[Ld], invL[Ls], np.asarray(L_values, np.float32))

    # A stage with self loops; dests in permA space.
    sl = np.arange(N, dtype=np.int64)
    Ad = invA[np.concatenate([Acol, sl])]
    As_nodes = np.concatenate([Arow, sl])
    stA1 = _build_stage(Ad, invL[As_nodes], None)   # conv1 gathers permL table
    stA23 = _build_stage(Ad, invA[As_nodes], None)  # conv2/3 gather permA tables

    # node-feature marshalling
    Xp = np.zeros((NTOT, DIN), np.float32)
    mask = permL >= 0
    Xp[mask] = np.asarray(X, np.float32)[permL[mask]]
    XT = np.ascontiguousarray(Xp.T)                       # [128, NTOT]

    disL = np.zeros((NTOT, 1), np.float32)
    disL[mask, 0] = dis[permL[mask]]
    maskA = permA >= 0
    disA = np.zeros((NTOT, 1), np.float32)
    disA[maskA, 0] = dis[permA[maskA]]
    batchA = np.full((NTOT, 1), -1.0, np.float32)
    batchA[maskA, 0] = np.asarray(batch, np.float32)[permA[maskA]]

    # weights
    W1 = np.asarray(W1, np.float32)
    W1a, W1b, W1c = W1[:DIN], W1[DIN:2 * DIN], W1[2 * DIN:]
    W1ac = np.ascontiguousarray(W1a - W1c)                # [128, 64]
    W1bc = np.ascontiguousarray(np.concatenate([W1b, W1c], axis=1))  # [128,128]

    counts = np.bincount(np.asarray(batch, np.int64), minlength=NG).astype(np.float64)
    inv3n = (1.0 / (3.0 * np.maximum(counts, 1.0))).astype(np.float32)[:, None]

    grid = np.broadcast_to(np.arange(NG, dtype=np.float32)[None, :], (P, NG)).copy()

    rep = dict(
        XT=XT,
        W1ac=W1ac, W1bc=W1bc,
        W2=np.asarray(W2, np.float32), W3=np.asarray(W3, np.float32),
        Wout=np.asarray(Wout, np.float32),
        b1r=np.tile(np.asarray(b1, np.float32)[None, :], (P, 1)),
        b2r=np.tile(np.asarray(b2, np.float32)[None, :], (P, 1)),
        b3r=np.tile(np.asarray(b3, np.float32)[None, :], (P, 1)),
        boutr=np.tile(np.asarray(bout, np.float32)[None, :], (P, 1)),
        grid=grid, inv3n=inv3n,
        ident_in=np.eye(P, dtype=np.float32),
        dummy_tab=np.zeros((P, F), np.float32),
        dummy_idx=np.zeros((P, 8), np.int16),
    )

    in_maps = []
    for c in range(NCORES):
        r0 = c * NPC
        m = dict(rep)
        m["XTOWN"] = np.ascontiguousarray(XT[:, r0:r0 + NPC])
        m["disL"] = disL[r0:r0 + NPC].copy()
        m["disA"] = disA[r0:r0 + NPC].copy()
        m["batchA"] = batchA[r0:r0 + NPC].copy()
        m["IDXL_LO"] = stL["idx_lo"][c]
        m["IDXL_HI"] = stL["idx_hi"][c]
        m["VL_LO"] = stL["v_lo"][c]
        m["VL_HI"] = stL["v_hi"][c]
        m["IDXA1_LO"] = stA1["idx_lo"][c]
        m["IDXA1_HI"] = stA1["idx_hi"][c]
        m["IDXA23_LO"] = stA23["idx_lo"][c]
        m["IDXA23_HI"] = stA23["idx_hi"][c]
        in_maps.append(m)

    meta = {"stL": stL, "stA1": stA1, "stA23": stA23}
    return in_maps, meta


# ---------------- device program ----------------

_GQ = [0]


def _emit_gather(nc, sb, gtag, cap_elems, table_view, idx_dram, deg_cums,
                 chunk, W, bufname):
    """Emit one dma_gather for tiles [b0,b1) of a stage half.

    Returns (tile_ap, base_slot) so callers can slice per-tile regions."""
    b0, b1 = chunk
    nslots = int(deg_cums[b1] - deg_cums[b0])
    if nslots == 0:
        return None, 0
    nidx = nslots * P
    idx_sb = sb.tile([P, nidx // 16], I16, tag=f"{gtag}_idx", bufs=2,
                     name=f"{bufname}_idx")
    col0 = int(deg_cums[b0]) * (P // 16)
    nc.sync.dma_start(out=idx_sb[:, :], in_=idx_dram[:, col0:col0 + nidx // 16])
    g = sb.tile([P, cap_elems * W], F32, tag=gtag, bufs=2, name=bufname)
    nc.gpsimd.dma_gather(
        out_ap=g[:, :nslots * W].rearrange("p (n w) -> p n w", w=W),
        in_ap=table_view,
        idxs_ap=idx_sb[:, :],
        num_idxs=nidx,
        num_idxs_reg=nidx,
        elem_size=W,
        single_packet=False,
    )
    return g, int(deg_cums[b0])


def _build_program(meta):
    PHASES = int(os.environ.get("K_PHASES", "9"))
    stL, stA1, stA23 = meta["stL"], meta["stA1"], meta["stA23"]

    nc = bacc.Bacc("TRN2", target_bir_lowering=False, debug=False,
                   num_devices=NCORES)

    def din(name, shape, dt=F32):
        return nc.dram_tensor(name, shape, dt, kind="ExternalInput").ap()

    XT = din("XT", [P, NTOT])
    XTOWN = din("XTOWN", [P, NPC])
    W1ac = din("W1ac", [DIN, F])
    W1bc = din("W1bc", [DIN, DIN])
    W2 = din("W2", [F, F])
    W3 = din("W3", [F, F])
    Wout = din("Wout", [F, DOUT])
    b1r = din("b1r", [P, F])
    b2r = din("b2r", [P, F])
    b3r = din("b3r", [P, F])
    boutr = din("boutr", [P, DOUT])
    grid = din("grid", [P, NG])
    ident_in = din("ident_in", [P, P])
    dummy_tab = din("dummy_tab", [P, F])
    dummy_idx = din("dummy_idx", [P, 8], I16)
    inv3n = din("inv3n", [NG, 1])
    disL_d = din("disL", [NPC, 1])
    disA_d = din("disA", [NPC, 1])
    batchA_d = din("batchA", [NPC, 1])
    IDXL_LO = din("IDXL_LO", [P, stL["SL"] * 8], I16)
    IDXL_HI = din("IDXL_HI", [P, stL["SH"] * 8], I16)
    VL_LO = din("VL_LO", [P, stL["SL"]])
    VL_HI = din("VL_HI", [P, stL["SH"]])
    IDXA1_LO = din("IDXA1_LO", [P, stA1["SL"] * 8], I16)
    IDXA1_HI = din("IDXA1_HI", [P, stA1["SH"] * 8], I16)
    IDXA23_LO = din("IDXA23_LO", [P, stA23["SL"] * 8], I16)
    IDXA23_HI = din("IDXA23_HI", [P, stA23["SH"] * 8], I16)

    OUT = nc.dram_tensor("out", [NG, DOUT], F32, kind="ExternalOutput").ap()

    with tile.TileContext(nc) as tc:
        with (
            tc.tile_pool(name="dram", bufs=1, space="DRAM") as dr,
            tc.tile_pool(name="sbuf", bufs=1) as sb,
            tc.tile_pool(name="psum", bufs=1, space="PSUM") as ps,
        ):
            # ---- DRAM internal buffers ----
            bc_table = dr.tile([NTOT, DIN], F32, name="bc_table")
            lc_local = dr.tile([NPC, F], F32, name="lc_local")
            lc_table = dr.tile([NTOT, F], F32, addr_space="Shared",
                               name="lc_table")
            h_local = [dr.tile([NPC, F], F32, name=f"h{i}_local")
                       for i in (1, 2, 3)]
            h_table = [dr.tile([NTOT, F], F32, addr_space="Shared",
                               name=f"h{i}_table") for i in (1, 2, 3)]
            pp_local = dr.tile([F, NG], F32, name="pp_local")
            pp_full = dr.tile([F, NG], F32, addr_space="Shared", name="pp_full")

            # ---- library prefetch: tiny gather so the Q7 mlp library
            # load overlaps the X-phase ----
            didx = sb.tile([P, 8], I16, name="didx")
            nc.sync.dma_start(out=didx[:, :], in_=dummy_idx[:, :])
            dg = sb.tile([P, F], F32, name="dg")
            nc.gpsimd.dma_gather(
                out_ap=dg[:].rearrange("p (n w) -> p n w", w=F),
                in_ap=dummy_tab[:, :], idxs_ap=didx[:, :],
                num_idxs=P, num_idxs_reg=P, elem_size=F,
                single_packet=False)

            # ---- constants / statics in SBUF ----
            ident = sb.tile([P, P], F32, name="ident")
            nc.sync.dma_start(out=ident[:, :], in_=ident_in[:, :])
            w1ac_sb = sb.tile([DIN, F], F32, name="w1ac_sb")
            nc.sync.dma_start(out=w1ac_sb[:, :], in_=W1ac[:, :])
            w1bc_sb = sb.tile([DIN, DIN], F32, name="w1bc_sb")
            nc.sync.dma_start(out=w1bc_sb[:, :], in_=W1bc[:, :])
            w2_sb = sb.tile([F, F], F32, name="w2_sb")
            nc.sync.dma_start(out=w2_sb[:, :], in_=W2[:, :])
            w3_sb = sb.tile([F, F], F32, name="w3_sb")
            nc.sync.dma_start(out=w3_sb[:, :], in_=W3[:, :])
            wout_sb = sb.tile([F, DOUT], F32, name="wout_sb")
            nc.sync.dma_start(out=wout_sb[:, :], in_=Wout[:, :])
            b_sb = []
            for nm, t in (("b1r", b1r), ("b2r", b2r), ("b3r", b3r)):
                bb = sb.tile([P, F], F32, name=f"{nm}_sb")
                nc.sync.dma_start(out=bb[:, :], in_=t[:, :])
                b_sb.append(bb)
            boutr_sb = sb.tile([P, DOUT], F32, name="boutr_sb")
            nc.sync.dma_start(out=boutr_sb[:, :], in_=boutr[:, :])
            grid_sb = sb.tile([P, NG], F32, name="grid_sb")
            nc.sync.dma_start(out=grid_sb[:, :], in_=grid[:, :])
            inv3n_sb = sb.tile([P, 4], F32, name="inv3n_sb")
            nc.sync.dma_start(out=inv3n_sb[:, :],
                              in_=inv3n[:].rearrange("(c p) o -> p (c o)", p=P))
            disL_sb = sb.tile([P, TPC], F32, name="disL_sb")
            nc.sync.dma_start(out=disL_sb[:, :],
                              in_=disL_d[:].rearrange("(b p) o -> p (b o)", p=P))
            disA_sb = sb.tile([P, TPC], F32, name="disA_sb")
            nc.sync.dma_start(out=disA_sb[:, :],
                              in_=disA_d[:].rearrange("(b p) o -> p (b o)", p=P))
            batch_sb = sb.tile([P, TPC], F32, name="batch_sb")
            nc.sync.dma_start(out=batch_sb[:, :],
                              in_=batchA_d[:].rearrange("(b p) o -> p (b o)", p=P))
            vlo_sb = sb.tile([P, stL["SL"]], F32, name="vlo_sb")
            nc.sync.dma_start(out=vlo_sb[:, :], in_=VL_LO[:, :])
            vhi_sb = sb.tile([P, stL["SH"]], F32, name="vhi_sb")
            nc.sync.dma_start(out=vhi_sb[:, :], in_=VL_HI[:, :])

            # persistent accumulators
            h1acc = sb.tile([P, TPC * F], F32, name="h1acc")
            x1_all = sb.tile([P, TPC * F], F32, name="x1_all")
            x2_all = sb.tile([P, TPC * F], F32, name="x2_all")

            # ---- phase 0a: BC table (replicated full compute, 4x batch) ----
            XB = 4
            for t4 in range(NTOT // (P * XB)):
                xt = sb.tile([P, XB * P], F32, tag="xph", bufs=3, name="xt")
                nc.sync.dma_start(
                    out=xt[:, :], in_=XT[:, t4 * XB * P:(t4 + 1) * XB * P])
                bcs = sb.tile([P, XB * DIN], F32, tag="bcs", bufs=3, name="bcs")
                for j in range(XB):
                    pm = ps.tile([P, DIN], F32, tag="ps_m", bufs=3, name="pm")
                    nc.tensor.matmul(out=pm[:, :],
                                     lhsT=xt[:, j * P:(j + 1) * P],
                                     rhs=w1bc_sb[:, :], start=True, stop=True)
                    nc.scalar.copy(out=bcs[:, j * DIN:(j + 1) * DIN],
                                   in_=pm[:, :])
                nc.sync.dma_start(
                    out=bc_table[t4 * XB * P:(t4 + 1) * XB * P, :]
                    .rearrange("(j p) f -> p j f", p=P),
                    in_=bcs[:].rearrange("p (j f) -> p j f", j=XB))

            # ---- phase 0b: A0 into h1acc (own shard only, 4x batch) ----
            for t4 in range((TPC + 3) // 4):
                bs = [b for b in range(t4 * 4, min(t4 * 4 + 4, TPC))]
                xt = sb.tile([P, XB * P], F32, tag="xph", bufs=3, name="xto")
                nc.sync.dma_start(
                    out=xt[:, :len(bs) * P],
                    in_=XTOWN[:, bs[0] * P:(bs[-1] + 1) * P])
                for j, b in enumerate(bs):
                    pm = ps.tile([P, F], F32, tag="ps_m", bufs=3, name="pma")
                    nc.tensor.matmul(out=pm[:, :],
                                     lhsT=xt[:, j * P:(j + 1) * P],
                                     rhs=w1ac_sb[:, :], start=True, stop=True)
                    nc.scalar.copy(out=h1acc[:, b * F:(b + 1) * F],
                                   in_=pm[:, :])

            # ---- generic spmm stage runner ----
            def run_stage(st, idx_lo_d, idx_hi_d, table, W, caps, per_tile):
                cap_lo, cap_hi = caps
                lo_chunks = _chunks(st["DEG_LO"], cap_lo)
                hi_chunks = _chunks(st["DEG_HI"], cap_hi)
                lo_i = hi_i = 0
                g_lo = g_hi = None
                base_lo = base_hi = 0
                for b in range(TPC):
                    if lo_i < len(lo_chunks) and lo_chunks[lo_i][0] == b:
                        g_lo, base_lo = _emit_gather(
                            nc, sb, "glo", cap_lo, table[0:VIEW, :],
                            idx_lo_d, st["CUM_LO"], lo_chunks[lo_i], W, "glo")
                        lo_i += 1
                    if hi_i < len(hi_chunks) and hi_chunks[hi_i][0] == b:
                        g_hi, base_hi = _emit_gather(
                            nc, sb, "ghi", cap_hi, table[HI_BASE:NTOT, :],
                            idx_hi_d, st["CUM_HI"], hi_chunks[hi_i], W, "ghi")
                        hi_i += 1
                    dlo = int(st["DEG_LO"][b])
                    dhi = int(st["DEG_HI"][b])
                    olo = (int(st["CUM_LO"][b]) - base_lo) * W
                    ohi = (int(st["CUM_HI"][b]) - base_hi) * W
                    per_tile(b, g_lo, olo, dlo, g_hi, ohi, dhi)

            NOVMULT = bool(int(os.environ.get("K_NOVMULT", "0")))

            def seg_reduce(out_ap, g, off, deg, W, vb=None):
                """out += nothing; writes sum over deg slots of g[:, off:off+deg*W]."""
                blk = g[:, off:off + deg * W]
                if NOVMULT:
                    vb = None
                if vb is not None:
                    nc.vector.tensor_tensor(
                        out=blk.rearrange("p (j w) -> p j w", w=W),
                        in0=blk.rearrange("p (j w) -> p j w", w=W),
                        in1=vb.to_broadcast([P, deg, W]),
                        op=mybir.AluOpType.mult)
                nc.vector.tensor_reduce(
                    out=out_ap, in_=blk.rearrange("p (j w) -> p w j", w=W),
                    axis=mybir.AxisListType.X, op=mybir.AluOpType.add)

            # ---- phase 1: spmm1  (gather BC -> LB0|LC0) ----
            GONLY = bool(int(os.environ.get("K_GONLY", "0")))

            def spmm1_tile(b, g_lo, olo, dlo, g_hi, ohi, dhi):
                if GONLY:
                    return
                cl = int(stL["CUM_LO"][b])
                ch = int(stL["CUM_HI"][b])
                rlo = sb.tile([P, DIN], F32, tag="red128", bufs=3, name="rlo1")
                seg_reduce(rlo[:, :], g_lo, olo, dlo, DIN,
                           vb=vlo_sb[:, cl:cl + dlo])
                rhi = sb.tile([P, DIN], F32, tag="red128", bufs=3, name="rhi1")
                seg_reduce(rhi[:, :], g_hi, ohi, dhi, DIN,
                           vb=vhi_sb[:, ch:ch + dhi])
                # LB0 into h1acc (h1acc currently holds A0)
                nc.vector.tensor_add(out=h1acc[:, b * F:(b + 1) * F],
                                     in0=h1acc[:, b * F:(b + 1) * F],
                                     in1=rlo[:, 0:F])
                nc.vector.tensor_add(out=h1acc[:, b * F:(b + 1) * F],
                                     in0=h1acc[:, b * F:(b + 1) * F],
                                     in1=rhi[:, 0:F])
                # LC0 out to DRAM
                lcs = sb.tile([P, F], F32, tag="lcs", bufs=3, name="lcs")
                nc.vector.tensor_add(out=lcs[:, :], in0=rlo[:, F:DIN],
                                     in1=rhi[:, F:DIN])
                nc.sync.dma_start(out=lc_local[b * P:(b + 1) * P, :],
                                  in_=lcs[:, :])

            if PHASES >= 1:
                run_stage(stL, IDXL_LO, IDXL_HI, bc_table, DIN,
                          (CAP_LO_W128, CAP_HI_W128), spmm1_tile)
            if PHASES >= 2:
                nc.gpsimd.collective_compute(
                    "AllGather", mybir.AluOpType.bypass,
                    replica_groups=[list(range(NCORES))],
                    ins=[lc_local[:, :]], outs=[lc_table[:, :]])

            # ---- phase 2: spmm2 (gather LC0 -> LLC0), finish h1s ----
            def spmm2_tile(b, g_lo, olo, dlo, g_hi, ohi, dhi):
                rlo = sb.tile([P, F], F32, tag="red64", bufs=3, name="rlo2")
                cl = int(stL["CUM_LO"][b])
                ch = int(stL["CUM_HI"][b])
                seg_reduce(rlo[:, :], g_lo, olo, dlo, F,
                           vb=vlo_sb[:, cl:cl + dlo])
                rhi = sb.tile([P, F], F32, tag="red64", bufs=3, name="rhi2")
                seg_reduce(rhi[:, :], g_hi, ohi, dhi, F,
                           vb=vhi_sb[:, ch:ch + dhi])
                nc.vector.tensor_add(out=rlo[:, :], in0=rlo[:, :], in1=rhi[:, :])
                # h1 += 2 * LLC0
                nc.vector.scalar_tensor_tensor(
                    out=h1acc[:, b * F:(b + 1) * F], in0=rlo[:, :], scalar=2.0,
                    in1=h1acc[:, b * F:(b + 1) * F],
                    op0=mybir.AluOpType.mult, op1=mybir.AluOpType.add)
                # h1s = dis * h1
                hs = sb.tile([P, F], F32, tag="hs", bufs=3, name="h1s")
                nc.vector.tensor_scalar_mul(out=hs[:, :],
                                            in0=h1acc[:, b * F:(b + 1) * F],
                                            scalar1=disL_sb[:, b:b + 1])
                nc.sync.dma_start(out=h_local[0][b * P:(b + 1) * P, :],
                                  in_=hs[:, :])

            if PHASES >= 3:
                run_stage(stL, IDXL_LO, IDXL_HI, lc_table, F,
                          (CAP_LO_W64, CAP_HI_W64), spmm2_tile)
            if PHASES >= 4:
                nc.gpsimd.collective_compute(
                    "AllGather", mybir.AluOpType.bypass,
                    replica_groups=[list(range(NCORES))],
                    ins=[h_local[0][:, :]], outs=[h_table[0][:, :]])

            # ---- phases 3-5: convs ----
            pool_ps = ps.tile([F, NG], F32, tag="ps_pool", bufs=1,
                              name="pool_ps")
            pool_n = [0]  # number of pool matmuls emitted (147 total)

            def emit_pool_mm(x_tile_ap, ind_ap):
                i = pool_n[0]
                nc.tensor.matmul(out=pool_ps[:, :], lhsT=x_tile_ap, rhs=ind_ap,
                                 start=(i == 0), stop=(i == 3 * TPC - 1))
                pool_n[0] = i + 1

            def conv_tile(layer, b, g_lo, olo, dlo, g_hi, ohi, dhi):
                rlo = sb.tile([P, F], F32, tag="red64", bufs=3, name="rloC")
                seg_reduce(rlo[:, :], g_lo, olo, dlo, F)
                rhi = sb.tile([P, F], F32, tag="red64", bufs=3, name="rhiC")
                seg_reduce(rhi[:, :], g_hi, ohi, dhi, F)
                nc.vector.tensor_add(out=rlo[:, :], in0=rlo[:, :], in1=rhi[:, :])
                # x = relu(dis*S + b)
                xpre = sb.tile([P, F], F32, tag="xpre", bufs=3, name="xpre")
                nc.vector.scalar_tensor_tensor(
                    out=xpre[:, :], in0=rlo[:, :],
                    scalar=disA_sb[:, b:b + 1], in1=b_sb[layer - 1][:, :],
                    op0=mybir.AluOpType.mult, op1=mybir.AluOpType.add)
                if layer == 1:
                    xt_ap = x1_all[:, b * F:(b + 1) * F]
                elif layer == 2:
                    xt_ap = x2_all[:, b * F:(b + 1) * F]
                else:
                    x3t = sb.tile([P, F], F32, tag="x3t", bufs=3, name="x3t")
                    xt_ap = x3t[:, :]
                nc.scalar.activation(out=xt_ap, in_=xpre[:, :],
                                     func=mybir.ActivationFunctionType.Relu)
                if layer < 3:
                    # h_{l+1}s = (dis*x) @ W_{l+1}
                    xs = sb.tile([P, F], F32, tag="xs", bufs=3, name="xs")
                    nc.vector.tensor_scalar_mul(out=xs[:, :], in0=xt_ap,
                                                scalar1=disA_sb[:, b:b + 1])
                    tp = ps.tile([F, P], F32, tag="ps_t", bufs=2, name="tp")
                    nc.tensor.transpose(out=tp[:, :], in_=xs[:, :],
                                        identity=ident[:])
                    xsT = sb.tile([F, P], F32, tag="xsT", bufs=3, name="xsT")
                    nc.vector.tensor_copy(out=xsT[:, :], in_=tp[:, :])
                    hm = ps.tile([P, F], F32, tag="ps_m", bufs=3, name="hm")
                    wnext = w2_sb if layer == 1 else w3_sb
                    nc.tensor.matmul(out=hm[:, :], lhsT=xsT[:, :],
                                     rhs=wnext[:, :], start=True, stop=True)
                    hs = sb.tile([P, F], F32, tag="hs", bufs=3, name="hls")
                    nc.scalar.copy(out=hs[:, :], in_=hm[:, :])
                    nc.sync.dma_start(out=h_local[layer][b * P:(b + 1) * P, :],
                                      in_=hs[:, :])
                else:
                    # pooling contributions for x1, x2, x3 of this tile
                    ind = sb.tile([P, NG], F32, tag="ind", bufs=2, name="ind")
                    nc.vector.tensor_tensor(
                        out=ind[:, :],
                        in0=batch_sb[:, b:b + 1].to_broadcast([P, NG]),
                        in1=grid_sb[:, :], op=mybir.AluOpType.is_equal)
                    emit_pool_mm(x1_all[:, b * F:(b + 1) * F], ind[:, :])
                    emit_pool_mm(x2_all[:, b * F:(b + 1) * F], ind[:, :])
                    emit_pool_mm(xt_ap, ind[:, :])

            for layer in (1, 2, 3):
                if PHASES < 4 + layer:
                    break
                st = stA1 if layer == 1 else stA23
                ilo = IDXA1_LO if layer == 1 else IDXA23_LO
                ihi = IDXA1_HI if layer == 1 else IDXA23_HI
                run_stage(st, ilo, ihi, h_table[layer - 1], F,
                          (CAP_LO_W64, CAP_HI_W64),
                          lambda b, gl, ol, dl, gh, oh, dh, L=layer:
                          conv_tile(L, b, gl, ol, dl, gh, oh, dh))
                if layer < 3:
                    nc.gpsimd.collective_compute(
                        "AllGather", mybir.AluOpType.bypass,
                        replica_groups=[list(range(NCORES))],
                        ins=[h_local[layer][:, :]],
                        outs=[h_table[layer][:, :]])

            # ---- phase 6: pool partials -> AllReduce ----
            if PHASES < 8:
                zot = sb.tile([P, DOUT], F32, name="zot")
                nc.vector.memset(zot[:, :], 0)
                for c4 in range(4):
                    nc.sync.dma_start(out=OUT[c4 * P:(c4 + 1) * P, :],
                                      in_=zot[:, :])
            do_head = PHASES >= 8
            pool_sb = sb.tile([F, NG], F32, name="pool_sb")
            if do_head:
                nc.vector.tensor_copy(out=pool_sb[:, :], in_=pool_ps[:, :])
                nc.sync.dma_start(out=pp_local[:, :], in_=pool_sb[:, :])
                nc.gpsimd.collective_compute(
                    "AllReduce", mybir.AluOpType.add,
                    replica_groups=[list(range(NCORES))],
                    ins=[pp_local[:, :]], outs=[pp_full[:, :]])

                # ---- phase 7: head ----
                pp_sb = sb.tile([F, NG], F32, name="pp_sb")
                nc.sync.dma_start(out=pp_sb[:, :], in_=pp_full[:, :])
                zt_ps = ps.tile([DOUT, NG], F32, tag="ps_z", bufs=1, name="zt_ps")
                nc.tensor.matmul(out=zt_ps[:, :], lhsT=wout_sb[:, :],
                                 rhs=pp_sb[:, :], start=True, stop=True)
                zt_sb = sb.tile([DOUT, NG], F32, name="zt_sb")
                nc.vector.tensor_copy(out=zt_sb[:, :], in_=zt_ps[:, :])
                for c4 in range(4):
                    tr = ps.tile([P, DOUT], F32, tag="ps_t", bufs=2, name="tr")
                    nc.tensor.transpose(out=tr[:, :],
                                        in_=zt_sb[:, c4 * P:(c4 + 1) * P],
                                        identity=ident[:DOUT, :DOUT])
                    y = sb.tile([P, DOUT], F32, tag="ysm", bufs=2, name="y")
                    nc.vector.scalar_tensor_tensor(
                        out=y[:, :], in0=tr[:, :], scalar=inv3n_sb[:, c4:c4 + 1],
                        in1=boutr_sb[:, :],
                        op0=mybir.AluOpType.mult, op1=mybir.AluOpType.add)
                    mx = sb.tile([P, 1], F32, tag="mx", bufs=2, name="mx")
                    nc.vector.tensor_reduce(out=mx[:, :], in_=y[:, :],
                                            axis=mybir.AxisListType.X,
                                            op=mybir.AluOpType.max)
                    nmx = sb.tile([P, 1], F32, tag="nmx", bufs=2, name="nmx")
                    nc.vector.tensor_scalar_mul(out=nmx[:, :], in0=mx[:, :],
                                                scalar1=-1.0)
                    ex = sb.tile([P, DOUT], F32, tag="ex", bufs=2, name="ex")
                    ssum = sb.tile([P, 1], F32, tag="ssum", bufs=2, name="ssum")
                    nc.scalar.activation(out=ex[:, :], in_=y[:, :],
                                         func=mybir.ActivationFunctionType.Exp,
                                         bias=nmx[:, :1], scale=1.0,
                                         accum_out=ssum[:, :1])
                    rs = sb.tile([P, 1], F32, tag="rs", bufs=2, name="rs")
                    nc.vector.reciprocal(out=rs[:, :], in_=ssum[:, :])
                    ot = sb.tile([P, DOUT], F32, tag="ot", bufs=2, name="ot")
                    nc.vector.tensor_scalar_mul(out=ot[:, :], in0=ex[:, :],
                                                scalar1=rs[:, :1])
                    nc.sync.dma_start(out=OUT[c4 * P:(c4 + 1) * P, :],
                                      in_=ot[:, :])

    nc.compile()
    return nc


# ---------------- public entry ----------------

def kernel(X, L_indices, L_values, batch,
           W1, b1, W2, b2, W3, b3, Wout, bout):
    global LAST_EXEC_TIME_NS
    assert X.shape == (N, DIN)
    in_maps, meta = _prep(np.asarray(X), np.asarray(L_indices),
                          np.asarray(L_values), np.asarray(batch),
                          np.asarray(W1), np.asarray(W2), np.asarray(W3),
                          np.asarray(Wout), np.asarray(b1), np.asarray(b2),
                          np.asarray(b3), np.asarray(bout))
    nc = _build_program(meta)
    res = run_bass_kernel_spmd(nc, in_maps, core_ids=list(range(NCORES)))
    LAST_EXEC_TIME_NS = res.exec_time_ns
    if res.exec_time_ns is not None:
        print(f"HW exec time: {res.exec_time_ns} ns")
    return res.results[0]["out"]

